# revision 37
# baseline (speedup 1.0000x reference)
"""Cost-volume block kernel for Trainium2 (8 NeuronCores, batch-sharded).

Computes, for c1/warp of shape [B, H, W, C] (B=8, H=192, W=640, C=32):
    cost[d] = mean_c( c1[..., c] * warp_shifted_by(d-2)[..., c] )   d in 0..4
    out     = concat([c1, cost_0..cost_4], axis=-1)                 # [B,H,W,37]

v2 strategy (DVE-roofline products, everything else hidden under them):
  - one batch per NeuronCore (8 cores), SPMD via run_bass_kernel_spmd.
  - HW floor: the 19.66M bf16 products per core run on DVE in 2x_1P packed
    mode at 245.8 Ge/s (~80 us).  GpSimd shares DVE's SBUF port (measured:
    offloading one offset slows DVE TTs by 71%) and PE cannot express
    shift-correlation (diagonal extraction is not AP-expressible), so the
    kernel keeps DVE saturated and hides DMA / ScalarE / TensorE (~64 us
    matmul stream) underneath.
  - row-pair interleaving: two image rows interleaved pixel-by-pixel in
    the free dim, so a shift by d pixels is 2d elements = 4d bytes -
    always 4B-aligned.  One fused 5-offset TT per quadrant stays packed
    and the v1 ScalarE shifted-copy disappears.
  - warp travels as fp8 e3m4 (randn fits +-15.5, 4 mantissa bits), upcast
    fp8->bf16 by ScalarE; c1 stays bf16.  Input DMA: 11.9 MB/core.
  - software pipelining with compute(P-1) emitted BEFORE load(P) so the
    TTs' conservative DMA-sem waits never reference the next pair's loads;
    upcasts run ~2 pairs ahead on ScalarE and are never queued behind
    PSUM evacuation.
  - TensorE reduces channels: sparse [128, 20] stationary (1/32 entries)
    contracts partitions; tile_position=(0, 32q) routes quadrant q; the 5
    offsets accumulate per psum region (start/stop).
  - PSUM is not DMA-accessible, so ScalarE evacuates PSUM -> SBUF bf16
    into per-quadrant tiles (a shared pair-tile creates a false tile-level
    WAR chain evac(q+1) -> store(q), ~4.6us per link); stores ride the
    sync HWDGE ring (gpsimd's queue is software-DGE and lags ~5us;
    ScalarE's budget of upcasts+evacs is ~64us and cannot absorb trigger
    time).  Stores cover only the 20 valid partitions per quadrant
    ([6,4,20,1280] bf16 = 1.23 MB).
  - drain: the last pair interleaves pair-4 evacs between its quadrants,
    and the final quadrant is split lo (banks 0-1) / hi (bank 2) with both
    evacs on the then-idle DVE, so the tail chain after the last multiply
    is just the 256-column hi piece.
"""

import sys

if "/opt/trn_rl_repo" not in sys.path:
    sys.path.insert(0, "/opt/trn_rl_repo")

import numpy as np
from ml_dtypes import bfloat16, float8_e3m4

# Problem constants (hardcoded per harness contract).
B, H, W, C = 8, 192, 640, 32
SR = 2                  # search range
NOFF = 2 * SR + 1       # 5 disparity offsets
OUTC = C + NOFF         # 37 output channels

NP = 6                  # pairs per core (32 rows each)
NQ = 4                  # quadrants per pair: q picks an 8-row octet
NJ = 2                  # interleaved rows per (q, r) slot
NR = 4                  # rows per (q, j): partition p = r*32 + c
WI = W * NJ             # 1280: interleaved elems per (q) block (c1/product)
WHAL = W + 2 * SR       # 644 pixels incl. halo
WIH = WHAL * NJ         # 1288: interleaved elems per (q) block (warp)
FW = NQ * WI            # 5120 free elems per c1 tile
FWP = NQ * WIH          # 5152 free elems per warp tile
M = NR * NOFF           # 20 psum partitions per quadrant: m = r*5 + d
# psum column chunks (1280 cols over 3 banks of 512 f32)
CHUNKS = [(0, 0, 512), (1, 0, 512), (2, 0, 256)]  # (bank, off, len)

_BUILT = None


def _build():
    """Build + schedule the per-core Bass program (shapes are per-core)."""
    global _BUILT
    if _BUILT is not None:
        return _BUILT

    import concourse.bacc as bacc
    import concourse.mybir as mybir
    import concourse.tile as tile

    f32 = mybir.dt.float32
    bf16 = mybir.dt.bfloat16
    fp8 = mybir.dt.float8e3
    nc = bacc.Bacc("TRN2", target_bir_lowering=False, debug=False)
    c1T = nc.dram_tensor("c1t", [NP, 128, FW], bf16, kind="ExternalInput").ap()
    wpT = nc.dram_tensor("wpt", [NP, 128, FWP], fp8, kind="ExternalInput").ap()
    wp0 = nc.dram_tensor("wpt0", [128, WIH], bf16,
                         kind="ExternalInput").ap()
    sON = nc.dram_tensor("sones", [128, NOFF * M], bf16,
                         kind="ExternalInput").ap()
    out = nc.dram_tensor("out", [NP, NQ, M, WI], bf16,
                         kind="ExternalOutput").ap()

    def _apv(t, off, dims):
        # AP on tile t: keep the partition dim, custom free dims at elem
        # offset off (element strides).
        a = t[:]
        APc = type(a)
        return APc(a.tensor, a.offset + off, [list(a.ap[0])] + dims)

    with tile.TileContext(nc) as tc:
        with tc.tile_pool(name="const", bufs=1) as cons, \
             tc.tile_pool(name="ins", bufs=3) as ins, \
             tc.tile_pool(name="prod", bufs=3) as pr, \
             tc.tile_pool(name="psum", bufs=2, space="PSUM") as pp, \
             tc.tile_pool(name="outs", bufs=3) as outs:
            s_t = cons.tile([128, NOFF * M], bf16)
            tiles = {}   # P -> (c1_t, wb_t)
            pend = []

            def _load(P):
                c1_t = ins.tile([128, FW], bf16, tag="c1")
                wb_t = ins.tile([128, FWP], bf16, tag="wb")
                w8_t = ins.tile([128, FWP], fp8, tag="w8")
                if P == 0:
                    # chunked per quadrant so the first TT starts as soon
                    # as the first quarter lands; q0's warp ships
                    # pre-upcast (bf16) so the head skips ScalarE, and the
                    # stationary rides right after (needed by the first
                    # matmul, ~4us later)
                    nc.sync.dma_start(out=wb_t[:, 0:648], in_=wp0[:, 0:648])
                    nc.sync.dma_start(out=c1_t[:, 0:512],
                                      in_=c1T[P][:, 0:512])
                    nc.sync.dma_start(out=s_t, in_=sON)
                    nc.sync.dma_start(out=wb_t[:, 648:WIH],
                                      in_=wp0[:, 648:WIH])
                    nc.sync.dma_start(out=c1_t[:, 512:WI],
                                      in_=c1T[P][:, 512:WI])
                    for q in range(1, NQ):
                        nc.sync.dma_start(
                            out=w8_t[:, q * WIH:(q + 1) * WIH],
                            in_=wpT[P][:, q * WIH:(q + 1) * WIH])
                        nc.sync.dma_start(out=c1_t[:, q * WI:(q + 1) * WI],
                                          in_=c1T[P][:, q * WI:(q + 1) * WI])
                else:
                    nc.sync.dma_start(out=w8_t, in_=wpT[P])
                    nc.sync.dma_start(out=c1_t, in_=c1T[P])
                for q in range(1 if P == 0 else 0, NQ):
                    nc.scalar.copy(out=wb_t[:, q * WIH:(q + 1) * WIH],
                                   in_=w8_t[:, q * WIH:(q + 1) * WIH])
                tiles[P] = (c1_t, wb_t)

            def _mm(P, ps, pd_t, q, dlist):
                for (bank, off, ln) in CHUNKS:
                    col0 = bank * 512 + off
                    for d in dlist:
                        nc.tensor.matmul(
                            ps[32 * q:32 * q + M, bank, off:off + ln],
                            s_t[:, d * M:(d + 1) * M],
                            pd_t[:, d * WI + col0:d * WI + col0 + ln],
                            start=(d == 0),
                            stop=(d == NOFF - 1),
                            tile_position=(0, 32 * q),
                        )

            def _tt_fused(c1_t, wb_t, pd_t, q):
                # one fused TT per quadrant: d is an outer AP dim with
                # stride 2 elems (4 bytes) into the interleaved warp block,
                # broadcast (stride 0) on c1; 2x_1P packed mode holds for
                # all 5 offsets at once
                nc.vector.tensor_mul(
                    _apv(pd_t, 0, [[WI, NOFF], [1, WI]]),
                    _apv(c1_t, q * WI, [[0, NOFF], [1, WI]]),
                    _apv(wb_t, q * WIH, [[2, NOFF], [1, WI]]))

            def _compute(P):
                c1_t, wb_t = tiles.pop(P)
                ps = pp.tile([128, 3, 512], f32, tag="ps", name=f"ps{P}")
                if P == 0:
                    # ramp: q0's first 512 columns form a small head piece
                    # so the first TT/matmuls start ~2us earlier, then the
                    # remaining columns and quadrants follow
                    pd_t = pr.tile([128, NOFF * WI], bf16, tag="pd")
                    nc.vector.tensor_mul(
                        _apv(pd_t, 0, [[WI, NOFF], [1, 512]]),
                        _apv(c1_t, 0, [[0, NOFF], [1, 512]]),
                        _apv(wb_t, 0, [[2, NOFF], [1, 512]]))
                    for d in range(NOFF):
                        nc.tensor.matmul(
                            ps[0:M, 0, 0:512], s_t[:, d * M:(d + 1) * M],
                            pd_t[:, d * WI:d * WI + 512],
                            start=(d == 0), stop=(d == NOFF - 1),
                            tile_position=(0, 0))
                    nc.vector.tensor_mul(
                        _apv(pd_t, 512, [[WI, NOFF], [1, 768]]),
                        _apv(c1_t, 512, [[0, NOFF], [1, 768]]),
                        _apv(wb_t, 512, [[2, NOFF], [1, 768]]))
                    for d in range(NOFF):
                        for (bank, off, ln) in CHUNKS[1:]:
                            col0 = bank * 512 + off
                            nc.tensor.matmul(
                                ps[0:M, bank, off:off + ln],
                                s_t[:, d * M:(d + 1) * M],
                                pd_t[:, d * WI + col0:d * WI + col0 + ln],
                                start=(d == 0), stop=(d == NOFF - 1),
                                tile_position=(0, 0))
                    for q in range(1, NQ):
                        pd_t = pr.tile([128, NOFF * WI], bf16, tag="pd")
                        _tt_fused(c1_t, wb_t, pd_t, q)
                        _mm(P, ps, pd_t, q, range(NOFF))
                else:
                    # steady state: one fused TT per HALF-pair (d and the
                    # 2-quadrant dim both folded into the AP) halves the
                    # per-instruction init overhead
                    for h in range(2):
                        pd_t = pr.tile([128, 2 * NOFF * WI], bf16, tag="pdh")
                        nc.vector.tensor_mul(
                            _apv(pd_t, 0, [[2 * WI, NOFF], [WI, 2], [1, WI]]),
                            _apv(c1_t, 2 * h * WI,
                                 [[0, NOFF], [WI, 2], [1, WI]]),
                            _apv(wb_t, 2 * h * WIH,
                                 [[2, NOFF], [WIH, 2], [1, WI]]))
                        for qq in range(2):
                            q = 2 * h + qq
                            for (bank, off, ln) in CHUNKS:
                                col0 = bank * 512 + off
                                for d in range(NOFF):
                                    po = d * 2 * WI + qq * WI
                                    nc.tensor.matmul(
                                        ps[32 * q:32 * q + M, bank,
                                           off:off + ln],
                                        s_t[:, d * M:(d + 1) * M],
                                        pd_t[:, po + col0:po + col0 + ln],
                                        start=(d == 0),
                                        stop=(d == NOFF - 1),
                                        tile_position=(0, 32 * q),
                                    )
                pend.append((P, ps))

            def _evac_pair(P, ps):
                # one full-width evacuation per pair: ScalarE processes all
                # 128 partitions in parallel, so copying every quadrant band
                # at once costs the same ~1.6us as a single 20-partition
                # band did -- 4x less ScalarE time than per-quadrant evacs
                o_t = outs.tile([128, WI], bf16, tag="o")
                lo = o_t[:, 0:1024].rearrange("p (a b) -> p a b", a=2)
                nc.scalar.copy(out=lo, in_=ps[:, 0:2, 0:512])
                nc.scalar.copy(out=o_t[:, 1024:WI], in_=ps[:, 2, 0:256])
                # store triggers ride sync: the only spare HWDGE ring
                # (gpsimd's queue is software-DGE and lags ~5us)
                for q in range(NQ):
                    band = slice(32 * q, 32 * q + M)
                    nc.sync.dma_start(out=out[P, q], in_=o_t[band, 0:WI])

            def _evac(rec):
                P, ps = rec
                _evac_pair(P, ps)

            def _drain(P, prev_rec):
                """Last pair: split the final quadrant lo/hi so the tail
                chain after the last multiply is just the 256-column hi
                piece (evacuated on the then-idle DVE)."""
                c1_t, wb_t = tiles.pop(P)
                ps = pp.tile([128, 3, 512], f32, tag="ps", name=f"ps{P}")
                pP, psP = prev_rec
                qlast = NQ - 1
                for q in range(NQ - 1):
                    pd_t = pr.tile([128, NOFF * WI], bf16, tag="pd")
                    _tt_fused(c1_t, wb_t, pd_t, q)
                    _mm(P, ps, pd_t, q, range(NOFF))
                # final quadrant, lo columns (banks 0-1) first
                pd_t = pr.tile([128, NOFF * WI], bf16, tag="pd")
                nc.vector.tensor_mul(
                    _apv(pd_t, 0, [[WI, NOFF], [1, 1024]]),
                    _apv(c1_t, qlast * WI, [[0, NOFF], [1, 1024]]),
                    _apv(wb_t, qlast * WIH, [[2, NOFF], [1, 1024]]))
                for d in range(NOFF):
                    for (bank, off, ln) in CHUNKS[:2]:
                        nc.tensor.matmul(
                            ps[32 * qlast:32 * qlast + M, bank, off:off + ln],
                            s_t[:, d * M:(d + 1) * M],
                            pd_t[:, d * WI + bank * 512:
                                 d * WI + bank * 512 + ln],
                            start=(d == 0), stop=(d == NOFF - 1),
                            tile_position=(0, 32 * qlast))
                _evac_pair(pP, psP)
                # pair-5 partial evac on ScalarE during the hi TTs: full lo
                # columns for all quadrants, hi columns for q0-2 only
                o_t = outs.tile([128, WI], bf16, tag="o")
                lo = o_t[:, 0:1024].rearrange("p (a b) -> p a b", a=2)
                nc.scalar.copy(out=lo, in_=ps[:, 0:2, 0:512])
                nc.scalar.copy(out=o_t[0:96, 1024:WI], in_=ps[0:96, 2, 0:256])
                for q in range(NQ - 1):
                    band = slice(32 * q, 32 * q + M)
                    eng = nc.sync if q % 2 == 0 else nc.scalar
                    eng.dma_start(out=out[P, q], in_=o_t[band, 0:WI])
                band3 = slice(32 * qlast, 32 * qlast + M)
                nc.scalar.dma_start(out=out[P, qlast][:, 0:1024],
                                    in_=o_t[band3, 0:1024])
                # final quadrant, hi columns (bank 2): the only work left
                # after the last big TT, so the tail is ~2.5us
                nc.vector.tensor_mul(
                    _apv(pd_t, 1024, [[WI, NOFF], [1, 256]]),
                    _apv(c1_t, qlast * WI + 1024, [[0, NOFF], [1, 256]]),
                    _apv(wb_t, qlast * WIH + 1024, [[2, NOFF], [1, 256]]))
                for d in range(NOFF):
                    nc.tensor.matmul(
                        ps[32 * qlast:32 * qlast + M, 2, 0:256],
                        s_t[:, d * M:(d + 1) * M],
                        pd_t[:, d * WI + 1024:d * WI + 1280],
                        start=(d == 0), stop=(d == NOFF - 1),
                        tile_position=(0, 32 * qlast))
                ohi = outs.tile([128, 256], bf16, tag="ohi")
                nc.vector.tensor_copy(ohi[band3, 0:256],
                                      ps[band3, 2, 0:256])
                nc.sync.dma_start(out=out[P, qlast][:, 1024:WI],
                                  in_=ohi[band3, 0:256])

            # software-pipelined main loop; _compute(P-1) is emitted
            # BEFORE _load(P) so the TTs' conservative DMA-sem waits never
            # reference the next pair's loads
            _load(0)
            for P in range(1, NP):
                _compute(P - 1)
                _load(P)
                if P >= 2:
                    _evac(pend.pop(0))
            _drain(NP - 1, pend.pop(0))

    nc.compile()
    _BUILT = nc
    return _BUILT


def _prep_c1(c1):
    """[B, H, W, C] f32 -> [B, NP, 128, FW] bf16, row-pair interleaved.

    row = P*32 + q*8 + j*4 + r; partition = r*32 + c; free = q*1280 + 2w + j
    """
    t = c1.reshape(B, NP, NQ, NJ, NR, W, C)         # b P q j r w c
    t = t.transpose(0, 1, 4, 6, 2, 5, 3)            # b P r c q w j
    return np.ascontiguousarray(t.reshape(B, NP, 128, FW)).astype(bfloat16)


def _prep_warp(warp):
    """[B, H, W, C] f32 -> haloed interleaved [B, NP, 128, FWP] f32."""
    wp = np.zeros((B, H, WHAL, C), dtype=np.float32)
    wp[:, :, SR:SR + W] = warp
    t = wp.reshape(B, NP, NQ, NJ, NR, WHAL, C)      # b P q j r w' c
    t = t.transpose(0, 1, 4, 6, 2, 5, 3)            # b P r c q w' j
    return np.ascontiguousarray(t.reshape(B, NP, 128, FWP))


def _make_sones():
    """[128, 5*20] bf16 stationaries; S_d[(r,c), m] = 1/32 iff m == r*5+d."""
    S = np.zeros((128, NOFF * M), dtype=np.float32)
    for d in range(NOFF):
        for r in range(NR):
            S[r * C:(r + 1) * C, d * M + r * NOFF + d] = 1.0 / C
    return S.astype(bfloat16)


def _run(c1t_full, wpt_full, trace=False, **kw):
    from concourse.bass_utils import run_bass_kernel_spmd

    nc = _build()
    sones = _make_sones()
    in_maps = [{"c1t": c1t_full[i],
                "wpt": wpt_full[i].astype(float8_e3m4),
                "wpt0": wpt_full[i][0, :, 0:WIH].astype(bfloat16),
                "sones": sones}
               for i in range(B)]
    return run_bass_kernel_spmd(nc, in_maps, list(range(B)), trace=trace, **kw)


def kernel(c1, warp, search_range):
    assert int(search_range) == SR, f"kernel hardcodes search_range={SR}"
    c1 = np.ascontiguousarray(np.asarray(c1, dtype=np.float32))
    warp = np.ascontiguousarray(np.asarray(warp, dtype=np.float32))
    assert c1.shape == (B, H, W, C) and warp.shape == (B, H, W, C)
    r = _run(_prep_c1(c1), _prep_warp(warp))
    out = np.empty((B, H, W, OUTC), dtype=np.float32)
    out[..., :C] = c1
    for i in range(B):
        cost = np.asarray(r.results[i]["out"]).astype(np.float32)
        # [P, q, m=(r,d), e=(w,j)] -> rows P*32 + q*8 + j*4 + r, pixel w, d
        cost = cost.reshape(NP, NQ, NR, NOFF, W, NJ)
        cost = cost.transpose(0, 1, 5, 2, 4, 3)     # P q j r w d
        out[i, ..., C:] = cost.reshape(H, W, NOFF)
    return out
